# revision 25
# baseline (speedup 1.0000x reference)
"""AirTNN Trainium2 kernel (8 NeuronCores, SPMD + AllGather).

Computation (reference): 3 sequential "shifts", each
    x_up <- (upper_lp * fad_k) @ x_up + noise_k
    x_low <- (lower_lp * fad_k) @ x_low + noise_k   (same noise)
with fad_k ~ Rayleigh drawn from jax.random with a fixed key, and
noise_k = std_k * g_k where std_k depends on the running signal power of
x_up (batch 0) and g_k are fixed normal samples.  The output accumulates
per-shift projections x_up @ up_W[k].T + x_low @ low_W[k].T plus x @ h_W.T.

Strategy:
 - Host: reproduce the PRNG samples (Threefry is backend-deterministic),
   fold fading into the shift matrices, compute the noise stds from an
   fp32 replica, and rescale everything so device activations stay O(1)
   in fp16.  The shift-2 noise term folds into a host-side additive
   output correction.
 - Device: row-shard the (transposed) shift matrices over 8 cores, fp16
   matmuls accumulated in fp32 PSUM, one AllGather per (boundary,
   branch) kept split so each branch's collective overlaps the other
   branch's compute.  Structural changes vs the original baseline:
   * the last shift's projection weights are folded into the boundary-1
     transport payload (the producer multiplies its y1 slice by the
     scaled W2 while transposing), so shift 2's matmuls accumulate
     straight into the output PSUM and the post-AllGather tail is just
     the matmuls + output DMA (no stt / projection matmuls / shift-2
     noise load on the tail);
   * gathered reads run on the scalar HWDGE queue in two rank-halves so
     the consuming matmuls pipeline with the read, and their waits can't
     block the sync queue's A-stream FIFOs;
   * a 9-deep A-stream pool prefetches most of the 25MB of shift
     matrices before the collective phase, reducing HBM contention with
     the collective firmware's bounce traffic.
   (Measured dead ends this session: a 1-byte pre-barrier AllGather costs
   ~14us of serialized cc-stream time and saves nothing; merging the two
   boundary-0 AllGathers delays the up-branch data past the stream slack;
   a remote_dma SBUF->SBUF AllGather is ~3x slower than the collective
   firmware because SDMA pays ~0.7us per 1KB partition-row packet.)
"""

import os
import sys

import numpy as np

sys.path.insert(0, "/opt/trn_rl_repo")

NCORES = 8
N = 4096
C = 64
B = 2
K = 2                  # taps; K+1 shifts
NSHIFT = K + 1
R = N // NCORES        # 512 rows per core
C2 = C * B             # 128 (both batches side by side)
NJ = N // 128          # 32 contraction chunks
NQ = 2                 # A-stream DMA granularity: halves of a branch-shift
JPQ = NJ // NQ         # 16 chunks per half
NTERM = 2 * NSHIFT + 1 # projection terms
SNR_LIN = 10.0
CF_COMP_STD = 0.5

_compiled = {}
LAST_RESULTS = None    # BassKernelResults of the most recent device run


def _build_nc():
    import concourse.bacc as bacc
    import concourse.mybir as mybir
    import concourse.tile as tile

    fp16 = mybir.dt.float16
    fp32 = mybir.dt.float32
    u8 = mybir.dt.uint8

    nc = bacc.Bacc("TRN2", target_bir_lowering=False, debug=False,
                   num_devices=NCORES)

    # pre-tiled A stream: row block (2k+br)*128+p, col j*512+m
    a_p = nc.dram_tensor("a_p", [NSHIFT * 2 * 128, NJ * R], fp16,
                         kind="ExternalInput")
    x0 = nc.dram_tensor("x0", [128, NJ * C2], fp16, kind="ExternalInput")
    xt0 = nc.dram_tensor("xt0", [C2, R], fp16, kind="ExternalInput")
    nz = nc.dram_tensor("nz", [2 * C2, R], fp32, kind="ExternalInput")
    wc = nc.dram_tensor("wc", [NTERM * C2, C2], fp16, kind="ExternalInput")
    bt = nc.dram_tensor("bt", [2 * 128, 1], fp32, kind="ExternalInput")
    idn = nc.dram_tensor("idn", [128, 128], fp16, kind="ExternalInput")
    out_t = nc.dram_tensor("out_t", [C2, R], fp32, kind="ExternalOutput")

    # one collective per (boundary, branch): rank block [p, sub*128+c2].
    # Keeping them split preserves the branch-staggered pipeline (each
    # branch's AllGather overlaps the other branch's compute).
    cc_in = [[nc.dram_tensor(f"cc_in{k}{br}", [128, 4 * C2], fp16)
              for br in range(2)] for k in range(NSHIFT - 1)]
    cc_out = [[nc.dram_tensor(f"cc_out{k}{br}", [NCORES * 128, 4 * C2], fp16,
                              addr_space="Shared")
               for br in range(2)] for k in range(NSHIFT - 1)]

    rg = [list(range(NCORES))]

    with tile.TileContext(nc) as tc:
        with (
            tc.tile_pool(name="const", bufs=1) as constp,
            tc.tile_pool(name="apool", bufs=9) as apool,
            tc.tile_pool(name="xgpool", bufs=32) as xgpool,
            tc.tile_pool(name="ccsb", bufs=2) as ccsbp,
            tc.tile_pool(name="psum", bufs=2, space="PSUM") as psump,
            tc.tile_pool(name="psumt", bufs=2, space="PSUM") as psumtp,
            tc.tile_pool(name="psumo", bufs=1, space="PSUM") as psumop,
        ):
            # critical-path preload first: shift-0 stationary operand
            X0 = constp.tile([128, NJ * C2], fp16, tag="x0")
            nc.sync.dma_start(X0[:], x0[:])

            lazy = {}

            def const_load(tag, shape, dtype, src):
                if tag not in lazy:
                    t = constp.tile(shape, dtype, tag=tag)
                    if src is not None:
                        nc.sync.dma_start(t[:], src)
                    lazy[tag] = t
                return lazy[tag]

            po = psumop.tile([C2, R], fp32, tag="po")
            n_po = [0]

            def po_mm(lhsT, rhs, last=False):
                mm = nc.tensor.matmul(po[:], lhsT, rhs,
                                      start=(n_po[0] == 0), stop=last)
                n_po[0] += 1
                return mm

            xgt = {}
            for k in range(NSHIFT):
                is_last = k == NSHIFT - 1
                for br in range(2):
                    ps = po if is_last else psump.tile([C2, R], fp32)
                    row0 = (k * 2 + br) * 128
                    for q in range(NQ):
                        aq = apool.tile([128, JPQ * R], fp16)
                        nc.sync.dma_start(
                            aq[:], a_p[row0:row0 + 128,
                                       q * JPQ * R:(q + 1) * JPQ * R])
                        for jj in range(JPQ):
                            j = q * JPQ + jj
                            if k == 0:
                                lhsT = X0[:, j * C2:(j + 1) * C2]
                            else:
                                lhsT = xgt[(k, br)][j // 4][
                                    :, (j % 4) * C2:(j % 4 + 1) * C2]
                            rhs = aq[:, jj * R:(jj + 1) * R]
                            if is_last:
                                po_mm(lhsT, rhs)
                            else:
                                nc.tensor.matmul(ps[:], lhsT, rhs,
                                                 start=(j == 0),
                                                 stop=(j == NJ - 1))
                    if is_last:
                        continue
                    # y = beta_k * psum + noise'_k, cast to fp16
                    NZk = const_load(f"nz{k}", [C2, R], fp32,
                                     nz[k * C2:(k + 1) * C2, :])
                    BTk = const_load(f"bt{k}", [128, 1], fp32,
                                     bt[k * 128:(k + 1) * 128, :])
                    yt = constp.tile([C2, R], fp16, tag=f"y{k}{br}")
                    nc.vector.scalar_tensor_tensor(
                        yt[:], ps[:], BTk[:], NZk[:],
                        op0=mybir.AluOpType.mult, op1=mybir.AluOpType.add)
                    # transport layout: boundary 0 sends y0 (plain PE
                    # transpose); boundary 1 sends y1 pre-multiplied by the
                    # folded, scaled W2 so shift 2 accumulates into po
                    ccsb = ccsbp.tile([128, 4 * C2], fp16)
                    if k == 0:
                        ident = const_load("ident", [128, 128], fp16, idn[:])
                        for s in range(4):
                            pt = psumtp.tile([128, 128], fp16)
                            nc.tensor.transpose(
                                pt[:], yt[:, s * 128:(s + 1) * 128], ident[:])
                            nc.vector.tensor_copy(
                                ccsb[:, s * C2:(s + 1) * C2], pt[:])
                    else:
                        WV = const_load(
                            f"wc{4 + br}", [C2, C2], fp16,
                            wc[(4 + br) * C2:(5 + br) * C2, :])
                        for s in range(4):
                            pt = psumtp.tile([128, 128], fp32)
                            nc.tensor.matmul(pt[:],
                                             yt[:, s * 128:(s + 1) * 128],
                                             WV[:], start=True, stop=True)
                            nc.vector.tensor_copy(
                                ccsb[:, s * C2:(s + 1) * C2], pt[:])
                    # cc_in write + collective stay on the SWDGE/gpsimd
                    # queue; the gathered read goes on the scalar HWDGE
                    # queue (faster, and its wait can't block the sync
                    # queue's A-stream FIFOs)
                    nc.gpsimd.dma_start(cc_in[k][br][:], ccsb[:])
                    nc.gpsimd.collective_compute(
                        "AllGather", mybir.AluOpType.bypass,
                        replica_groups=rg,
                        ins=[cc_in[k][br][:]], outs=[cc_out[k][br][:]])
                    # gathered read per rank block (contiguous 128KB each) so
                    # the consuming matmuls pipeline with the reads: chunk
                    # j only waits for rank j//4's block, not the whole 1MB
                    ranks = []
                    for r in range(NCORES):
                        t = xgpool.tile([128, 4 * C2], fp16)
                        nc.scalar.dma_start(
                            t[:], cc_out[k][br][r * 128:(r + 1) * 128, :])
                        ranks.append(t)
                    xgt[(k + 1, br)] = ranks
                    # this shift's projection term (off the cc path)
                    WCt = const_load(
                        f"wc{2 * k + br}", [C2, C2], fp16,
                        wc[(2 * k + br) * C2:(2 * k + br + 1) * C2, :])
                    po_mm(WCt[:], yt[:])

            XT0 = const_load("xt0", [C2, R], fp16, xt0[:])
            WCh = const_load(f"wc{NTERM - 1}", [C2, C2], fp16,
                             wc[(NTERM - 1) * C2:NTERM * C2, :])
            po_mm(WCh[:], XT0[:], last=True)
            OT = constp.tile([C2, R], fp32, tag="ot")
            nc.vector.tensor_copy(OT[:], po[:])
            nc.sync.dma_start(out_t[:], OT[:])

    nc.compile()
    return nc


def _host_precompute(x, lower_lp, upper_lp, up_W, low_W, h_W):
    """PRNG reproduction + scaling; returns per-core input maps, G, and the
    host-side additive correction for the folded shift-2 noise."""
    import jax
    import jax.numpy as jnp

    cpu = jax.devices("cpu")[0]
    f32 = np.float32

    with jax.default_device(cpu):
        key = jax.random.key(1)
        keys = jax.random.split(key, NSHIFT)
        fads, gs = [], []
        for i in range(NSHIFT):
            kf, kn = jax.random.split(keys[i])
            kr, ki = jax.random.split(kf)
            re = jax.random.normal(kr, (N, N), jnp.float32) * CF_COMP_STD
            im = jax.random.normal(ki, (N, N), jnp.float32) * CF_COMP_STD
            fads.append(np.asarray(jnp.sqrt(re * re + im * im)))
            gs.append(np.asarray(jax.random.normal(kn, (N, C), jnp.float32)))

    # fp32 replica of the up-branch batch-0 chain -> noise stds and scales
    stds = []
    z = x[0].astype(f32)
    for i in range(NSHIFT):
        stds.append(f32(np.sqrt(np.mean(z * z) / SNR_LIN)))
        z = (upper_lp * fads[i]).astype(f32) @ z + stds[i] * gs[i]
    r_last = f32(np.sqrt(np.mean(z * z)))
    r = [f32(stds[i + 1] * np.sqrt(SNR_LIN)) for i in range(NSHIFT - 1)]
    r.append(r_last)
    r_in = f32(np.sqrt(np.mean(x[0].astype(f32) ** 2)))
    G = float(r[-1])

    # big shift matrices: (lp * fad).T, fp16, column-sliced per core and
    # pre-tiled partition-major: a_p[(2k+br)*128+p, j*512+m] = AT[j*128+p, dR+m]
    a_p_cores = [np.empty((NSHIFT * 2 * 128, NJ * R), np.float16)
                 for _ in range(NCORES)]
    for k in range(NSHIFT):
        for br, lp in ((0, upper_lp), (1, lower_lp)):
            at16 = np.ascontiguousarray((lp * fads[k]).T).astype(np.float16)
            row0 = (k * 2 + br) * 128
            for d in range(NCORES):
                blk = at16[:, d * R:(d + 1) * R]          # [N, R]
                a_p_cores[d][row0:row0 + 128, :] = (
                    blk.reshape(NJ, 128, R).transpose(1, 0, 2)
                       .reshape(128, NJ * R))

    # normalized input, both batches side by side: X[n, c2]
    Xn = np.empty((N, C2), np.float16)
    Xn[:, :C] = (x[0].astype(f32) / r_in).astype(np.float16)
    Xn[:, C:] = (x[1].astype(f32) / r_in).astype(np.float16)
    # SBUF layout [p, j*128 + c2] = X[j*128 + p, c2]
    x0_sb = np.ascontiguousarray(
        Xn.reshape(NJ, 128, C2).transpose(1, 0, 2).reshape(128, NJ * C2))

    # per-core transposed input slice for the h_W projection
    xt0_cores = [np.ascontiguousarray(Xn[d * R:(d + 1) * R, :].T)
                 for d in range(NCORES)]

    # per-core noise slices (shifts 0,1 only; shift-2 noise folds to host),
    # transposed + duplicated for both batches
    nz_cores = [np.empty((2 * C2, R), f32) for _ in range(NCORES)]
    for k in range(2):
        nT = np.ascontiguousarray(((stds[k] / r[k]) * gs[k]).astype(f32).T)
        for d in range(NCORES):
            sl = nT[:, d * R:(d + 1) * R]
            nz_cores[d][k * C2:k * C2 + C, :] = sl
            nz_cores[d][k * C2 + C:(k + 1) * C2, :] = sl

    # projection weights, scale-folded, blockdiag over the two batches.
    # terms 0..3: shift 0/1 projections; 4,5: folded W2 (transport
    # pre-transform, scale r1/G); 6: h_W
    wc_np = np.zeros((NTERM * C2, C2), np.float16)
    terms = [
        (f32(r[0] / G), up_W[0]), (f32(r[0] / G), low_W[0]),
        (f32(r[1] / G), up_W[1]), (f32(r[1] / G), low_W[1]),
        (f32(r[1] / G), up_W[2]), (f32(r[1] / G), low_W[2]),
        (f32(r_in / G), h_W),
    ]
    for ti, (scale, W) in enumerate(terms):
        blk = (scale * W.astype(f32)).T.astype(np.float16)  # [c, o]
        wc_np[ti * C2:ti * C2 + C, :C] = blk
        wc_np[ti * C2 + C:(ti + 1) * C2, C:] = blk

    # per-shift scale ratios beta_k = r_{k-1} / r_k as [128,1] blocks
    bt_np = np.empty((2 * 128, 1), f32)
    r_prev = r_in
    for k in range(2):
        bt_np[k * 128:(k + 1) * 128, 0] = f32(r_prev / r[k])
        r_prev = r[k]

    # host-side correction: the folded shift-2 matmul omits the shift-2
    # noise; out gets + std2 * g2 @ (W2_up + W2_low).T for both batches
    corr = (stds[2] * gs[2].astype(f32)) @ (
        up_W[2].astype(f32) + low_W[2].astype(f32)).T  # [N, C]

    in_maps = []
    for d in range(NCORES):
        in_maps.append({
            "a_p": a_p_cores[d],
            "x0": x0_sb,
            "xt0": xt0_cores[d],
            "nz": nz_cores[d],
            "wc": wc_np,
            "bt": bt_np,
            "idn": np.eye(128, dtype=np.float16),
        })
    return in_maps, G, corr


def kernel(x, lower_lp, upper_lp, up_W, low_W, h_W):
    global LAST_RESULTS
    from concourse.bass_utils import run_bass_kernel_spmd

    x = np.asarray(x, np.float32)
    lower_lp = np.asarray(lower_lp, np.float32)
    upper_lp = np.asarray(upper_lp, np.float32)
    up_W = np.asarray(up_W, np.float32)
    low_W = np.asarray(low_W, np.float32)
    h_W = np.asarray(h_W, np.float32)

    in_maps, G, corr = _host_precompute(
        x, lower_lp, upper_lp, up_W, low_W, h_W)

    if "nc" not in _compiled:
        _compiled["nc"] = _build_nc()
    nc = _compiled["nc"]

    trace = os.environ.get("AIRTNN_TRACE", "0") == "1"
    res = run_bass_kernel_spmd(nc, in_maps, list(range(NCORES)), trace=trace)
    LAST_RESULTS = res

    # out[b, d*R + m, o] = G * out_t_d[o + 64*b, m] + corr[d*R + m, o]
    out = np.empty((B, N, C), np.float32)
    for d in range(NCORES):
        ot = res.results[d]["out_t"]  # [C2, R] fp32
        for b in range(B):
            out[b, d * R:(d + 1) * R, :] = (
                ot[b * C:(b + 1) * C, :].T) * G + corr[d * R:(d + 1) * R, :]
    return out


# revision 27
# speedup vs baseline: 1.0572x; 1.0572x over previous
"""AirTNN Trainium2 kernel (8 NeuronCores, SPMD + AllGather).

Computation (reference): 3 sequential "shifts", each
    x_up <- (upper_lp * fad_k) @ x_up + noise_k
    x_low <- (lower_lp * fad_k) @ x_low + noise_k   (same noise)
with fad_k ~ Rayleigh drawn from jax.random with a fixed key, and
noise_k = std_k * g_k where std_k depends on the running signal power of
x_up (batch 0) and g_k are fixed normal samples.  The output accumulates
per-shift projections x_up @ up_W[k].T + x_low @ low_W[k].T plus x @ h_W.T.

Strategy:
 - Host: reproduce the PRNG samples (Threefry is backend-deterministic),
   fold fading into the shift matrices, compute the noise stds from an
   fp32 replica, and rescale everything so device activations stay O(1)
   in fp16.  The shift-2 noise term folds into a host-side additive
   output correction.
 - Device: row-shard the (transposed) shift matrices over 8 cores, fp16
   matmuls accumulated in fp32 PSUM, one AllGather per (boundary,
   branch) kept split so each branch's collective overlaps the other
   branch's compute.  Structural changes vs the original baseline:
   * the last shift's projection weights are folded into the boundary-1
     transport payload (the producer multiplies its y1 slice by the
     scaled W2 while transposing), so shift 2's matmuls accumulate
     straight into the output PSUM and the post-AllGather tail is just
     the matmuls + output DMA (no stt / projection matmuls / shift-2
     noise load on the tail);
   * gathered reads run on the scalar HWDGE queue in two rank-halves so
     the consuming matmuls pipeline with the read, and their waits can't
     block the sync queue's A-stream FIFOs;
   * a 9-deep A-stream pool prefetches most of the 25MB of shift
     matrices before the collective phase, reducing HBM contention with
     the collective firmware's bounce traffic.
   (Measured dead ends this session: a 1-byte pre-barrier AllGather costs
   ~14us of serialized cc-stream time and saves nothing; merging the two
   boundary-0 AllGathers delays the up-branch data past the stream slack;
   a remote_dma SBUF->SBUF AllGather is ~3x slower than the collective
   firmware because SDMA pays ~0.7us per 1KB partition-row packet.)
"""

import os
import sys

import numpy as np

sys.path.insert(0, "/opt/trn_rl_repo")

NCORES = 8
N = 4096
C = 64
B = 2
K = 2                  # taps; K+1 shifts
NSHIFT = K + 1
R = N // NCORES        # 512 rows per core
C2 = C * B             # 128 (both batches side by side)
NJ = N // 128          # 32 contraction chunks
NQ = 2                 # A-stream DMA granularity: halves of a branch-shift
JPQ = NJ // NQ         # 16 chunks per half
NTERM = 2 * NSHIFT + 1 # projection terms
SNR_LIN = 10.0
CF_COMP_STD = 0.5

_compiled = {}
LAST_RESULTS = None    # BassKernelResults of the most recent device run


def _build_nc():
    import concourse.bacc as bacc
    import concourse.mybir as mybir
    import concourse.tile as tile

    fp16 = mybir.dt.float16
    fp32 = mybir.dt.float32
    u8 = mybir.dt.uint8

    nc = bacc.Bacc("TRN2", target_bir_lowering=False, debug=False,
                   num_devices=NCORES)

    # pre-tiled A stream: row block (2k+br)*128+p, col j*512+m
    a_p = nc.dram_tensor("a_p", [NSHIFT * 2 * 128, NJ * R], fp16,
                         kind="ExternalInput")
    x0 = nc.dram_tensor("x0", [128, NJ * C2], fp16, kind="ExternalInput")
    xt0 = nc.dram_tensor("xt0", [C2, R], fp16, kind="ExternalInput")
    nz = nc.dram_tensor("nz", [2 * C2, R], fp32, kind="ExternalInput")
    wc = nc.dram_tensor("wc", [NTERM * C2, C2], fp16, kind="ExternalInput")
    bt = nc.dram_tensor("bt", [2 * 128, 1], fp32, kind="ExternalInput")
    idn = nc.dram_tensor("idn", [128, 128], fp16, kind="ExternalInput")
    out_t = nc.dram_tensor("out_t", [C2, R], fp32, kind="ExternalOutput")

    # one collective per (boundary, branch): rank block [p, sub*128+c2].
    # Keeping them split preserves the branch-staggered pipeline (each
    # branch's AllGather overlaps the other branch's compute).
    cc_in = [[nc.dram_tensor(f"cc_in{k}{br}", [128, 4 * C2], fp16)
              for br in range(2)] for k in range(NSHIFT - 1)]
    cc_out = [[nc.dram_tensor(f"cc_out{k}{br}", [NCORES * 128, 4 * C2], fp16,
                              addr_space="Shared")
               for br in range(2)] for k in range(NSHIFT - 1)]

    rg = [list(range(NCORES))]

    with tile.TileContext(nc) as tc:
        with (
            tc.tile_pool(name="const", bufs=1) as constp,
            tc.tile_pool(name="apool", bufs=9) as apool,
            tc.tile_pool(name="xgpool", bufs=32) as xgpool,
            tc.tile_pool(name="ccsb", bufs=2) as ccsbp,
            tc.tile_pool(name="psum", bufs=2, space="PSUM") as psump,
            tc.tile_pool(name="psumt", bufs=2, space="PSUM") as psumtp,
            tc.tile_pool(name="psumo", bufs=1, space="PSUM") as psumop,
        ):
            # critical-path preload first: shift-0 stationary operand
            X0 = constp.tile([128, NJ * C2], fp16, tag="x0")
            nc.sync.dma_start(X0[:], x0[:])

            lazy = {}

            def const_load(tag, shape, dtype, src):
                if tag not in lazy:
                    t = constp.tile(shape, dtype, tag=tag)
                    if src is not None:
                        nc.sync.dma_start(t[:], src)
                    lazy[tag] = t
                return lazy[tag]

            po = psumop.tile([C2, R], fp32, tag="po")
            n_po = [0]

            def po_mm(lhsT, rhs, last=False):
                mm = nc.tensor.matmul(po[:], lhsT, rhs,
                                      start=(n_po[0] == 0), stop=last)
                n_po[0] += 1
                return mm

            xgt = {}
            for k in range(NSHIFT):
                is_last = k == NSHIFT - 1
                for br in range(2):
                    ps = po if is_last else psump.tile([C2, R], fp32)
                    row0 = (k * 2 + br) * 128
                    for q in range(NQ):
                        aq = apool.tile([128, JPQ * R], fp16)
                        nc.sync.dma_start(
                            aq[:], a_p[row0:row0 + 128,
                                       q * JPQ * R:(q + 1) * JPQ * R])
                        for jj in range(JPQ):
                            j = q * JPQ + jj
                            if k == 0:
                                lhsT = X0[:, j * C2:(j + 1) * C2]
                            else:
                                lhsT = xgt[(k, br)][j // 4][
                                    :, (j % 4) * C2:(j % 4 + 1) * C2]
                            rhs = aq[:, jj * R:(jj + 1) * R]
                            if is_last:
                                po_mm(lhsT, rhs)
                            else:
                                nc.tensor.matmul(ps[:], lhsT, rhs,
                                                 start=(j == 0),
                                                 stop=(j == NJ - 1))
                    if is_last:
                        continue
                    # y = beta_k * psum + noise'_k, cast to fp16
                    NZk = const_load(f"nz{k}", [C2, R], fp32,
                                     nz[k * C2:(k + 1) * C2, :])
                    BTk = const_load(f"bt{k}", [128, 1], fp32,
                                     bt[k * 128:(k + 1) * 128, :])
                    yt = constp.tile([C2, R], fp16, tag=f"y{k}{br}")
                    nc.vector.scalar_tensor_tensor(
                        yt[:], ps[:], BTk[:], NZk[:],
                        op0=mybir.AluOpType.mult, op1=mybir.AluOpType.add)
                    # transport layout: boundary 0 sends y0 (plain PE
                    # transpose); boundary 1 sends y1 pre-multiplied by the
                    # folded, scaled W2 so shift 2 accumulates into po
                    ccsb = ccsbp.tile([128, 4 * C2], fp16)
                    if k == 0:
                        ident = const_load("ident", [128, 128], fp16, idn[:])
                        for s in range(4):
                            pt = psumtp.tile([128, 128], fp16)
                            nc.tensor.transpose(
                                pt[:], yt[:, s * 128:(s + 1) * 128], ident[:])
                            nc.vector.tensor_copy(
                                ccsb[:, s * C2:(s + 1) * C2], pt[:])
                    else:
                        WV = const_load(
                            f"wc{4 + br}", [C2, C2], fp16,
                            wc[(4 + br) * C2:(5 + br) * C2, :])
                        for s in range(4):
                            pt = psumtp.tile([128, 128], fp32)
                            nc.tensor.matmul(pt[:],
                                             yt[:, s * 128:(s + 1) * 128],
                                             WV[:], start=True, stop=True)
                            nc.vector.tensor_copy(
                                ccsb[:, s * C2:(s + 1) * C2], pt[:])
                    # cc_in write + collective stay on the SWDGE/gpsimd
                    # queue; the gathered read goes on the scalar HWDGE
                    # queue (faster, and its wait can't block the sync
                    # queue's A-stream FIFOs)
                    nc.gpsimd.dma_start(cc_in[k][br][:], ccsb[:])
                    nc.gpsimd.collective_compute(
                        "AllGather", mybir.AluOpType.bypass,
                        replica_groups=rg,
                        ins=[cc_in[k][br][:]], outs=[cc_out[k][br][:]])
                    # gathered read per rank block (contiguous 128KB each) so
                    # the consuming matmuls pipeline with the reads: chunk
                    # j only waits for rank j//4's block, not the whole 1MB
                    ranks = []
                    for r in range(NCORES):
                        t = xgpool.tile([128, 4 * C2], fp16)
                        nc.scalar.dma_start(
                            t[:], cc_out[k][br][r * 128:(r + 1) * 128, :])
                        ranks.append(t)
                    xgt[(k + 1, br)] = ranks
                    # this shift's projection term (off the cc path)
                    WCt = const_load(
                        f"wc{2 * k + br}", [C2, C2], fp16,
                        wc[(2 * k + br) * C2:(2 * k + br + 1) * C2, :])
                    po_mm(WCt[:], yt[:])

            XT0 = const_load("xt0", [C2, R], fp16, xt0[:])
            WCh = const_load(f"wc{NTERM - 1}", [C2, C2], fp16,
                             wc[(NTERM - 1) * C2:NTERM * C2, :])
            po_mm(WCh[:], XT0[:], last=True)
            OT = constp.tile([C2, R], fp32, tag="ot")
            nc.vector.tensor_copy(OT[:], po[:])
            nc.sync.dma_start(out_t[:], OT[:])

    nc.compile()
    return nc


def _host_precompute(x, lower_lp, upper_lp, up_W, low_W, h_W):
    """PRNG reproduction + scaling; returns per-core input maps, G, and the
    host-side additive correction for the folded shift-2 noise."""
    import jax
    import jax.numpy as jnp

    cpu = jax.devices("cpu")[0]
    f32 = np.float32

    with jax.default_device(cpu):
        key = jax.random.key(1)
        keys = jax.random.split(key, NSHIFT)
        fads, gs = [], []
        for i in range(NSHIFT):
            kf, kn = jax.random.split(keys[i])
            kr, ki = jax.random.split(kf)
            re = jax.random.normal(kr, (N, N), jnp.float32) * CF_COMP_STD
            im = jax.random.normal(ki, (N, N), jnp.float32) * CF_COMP_STD
            fads.append(np.asarray(jnp.sqrt(re * re + im * im)))
            gs.append(np.asarray(jax.random.normal(kn, (N, C), jnp.float32)))

    # fp32 replica of the up-branch batch-0 chain -> noise stds and scales
    stds = []
    z = x[0].astype(f32)
    for i in range(NSHIFT):
        stds.append(f32(np.sqrt(np.mean(z * z) / SNR_LIN)))
        z = (upper_lp * fads[i]).astype(f32) @ z + stds[i] * gs[i]
    r_last = f32(np.sqrt(np.mean(z * z)))
    r = [f32(stds[i + 1] * np.sqrt(SNR_LIN)) for i in range(NSHIFT - 1)]
    r.append(r_last)
    r_in = f32(np.sqrt(np.mean(x[0].astype(f32) ** 2)))
    G = float(r[-1])

    # big shift matrices: (lp * fad).T, fp16, column-sliced per core and
    # pre-tiled partition-major: a_p[(2k+br)*128+p, j*512+m] = AT[j*128+p, dR+m]
    a_p_cores = [np.empty((NSHIFT * 2 * 128, NJ * R), np.float16)
                 for _ in range(NCORES)]
    for k in range(NSHIFT):
        for br, lp in ((0, upper_lp), (1, lower_lp)):
            at16 = np.ascontiguousarray((lp * fads[k]).T).astype(np.float16)
            row0 = (k * 2 + br) * 128
            for d in range(NCORES):
                blk = at16[:, d * R:(d + 1) * R]          # [N, R]
                a_p_cores[d][row0:row0 + 128, :] = (
                    blk.reshape(NJ, 128, R).transpose(1, 0, 2)
                       .reshape(128, NJ * R))

    # normalized input, both batches side by side: X[n, c2]
    Xn = np.empty((N, C2), np.float16)
    Xn[:, :C] = (x[0].astype(f32) / r_in).astype(np.float16)
    Xn[:, C:] = (x[1].astype(f32) / r_in).astype(np.float16)
    # SBUF layout [p, j*128 + c2] = X[j*128 + p, c2]
    x0_sb = np.ascontiguousarray(
        Xn.reshape(NJ, 128, C2).transpose(1, 0, 2).reshape(128, NJ * C2))

    # per-core transposed input slice for the h_W projection
    xt0_cores = [np.ascontiguousarray(Xn[d * R:(d + 1) * R, :].T)
                 for d in range(NCORES)]

    # per-core noise slices (shifts 0,1 only; shift-2 noise folds to host),
    # transposed + duplicated for both batches
    nz_cores = [np.empty((2 * C2, R), f32) for _ in range(NCORES)]
    for k in range(2):
        nT = np.ascontiguousarray(((stds[k] / r[k]) * gs[k]).astype(f32).T)
        for d in range(NCORES):
            sl = nT[:, d * R:(d + 1) * R]
            nz_cores[d][k * C2:k * C2 + C, :] = sl
            nz_cores[d][k * C2 + C:(k + 1) * C2, :] = sl

    # projection weights, scale-folded, blockdiag over the two batches.
    # terms 0..3: shift 0/1 projections; 4,5: folded W2 (transport
    # pre-transform, scale r1/G); 6: h_W
    wc_np = np.zeros((NTERM * C2, C2), np.float16)
    terms = [
        (f32(r[0] / G), up_W[0]), (f32(r[0] / G), low_W[0]),
        (f32(r[1] / G), up_W[1]), (f32(r[1] / G), low_W[1]),
        (f32(r[1] / G), up_W[2]), (f32(r[1] / G), low_W[2]),
        (f32(r_in / G), h_W),
    ]
    for ti, (scale, W) in enumerate(terms):
        blk = (scale * W.astype(f32)).T.astype(np.float16)  # [c, o]
        wc_np[ti * C2:ti * C2 + C, :C] = blk
        wc_np[ti * C2 + C:(ti + 1) * C2, C:] = blk

    # per-shift scale ratios beta_k = r_{k-1} / r_k as [128,1] blocks
    bt_np = np.empty((2 * 128, 1), f32)
    r_prev = r_in
    for k in range(2):
        bt_np[k * 128:(k + 1) * 128, 0] = f32(r_prev / r[k])
        r_prev = r[k]

    # host-side correction: the folded shift-2 matmul omits the shift-2
    # noise; out gets + std2 * g2 @ (W2_up + W2_low).T for both batches
    corr = (stds[2] * gs[2].astype(f32)) @ (
        up_W[2].astype(f32) + low_W[2].astype(f32)).T  # [N, C]

    in_maps = []
    for d in range(NCORES):
        in_maps.append({
            "a_p": a_p_cores[d],
            "x0": x0_sb,
            "xt0": xt0_cores[d],
            "nz": nz_cores[d],
            "wc": wc_np,
            "bt": bt_np,
            "idn": np.eye(128, dtype=np.float16),
        })
    return in_maps, G, corr


def kernel(x, lower_lp, upper_lp, up_W, low_W, h_W):
    global LAST_RESULTS
    from concourse.bass_utils import run_bass_kernel_spmd

    x = np.asarray(x, np.float32)
    lower_lp = np.asarray(lower_lp, np.float32)
    upper_lp = np.asarray(upper_lp, np.float32)
    up_W = np.asarray(up_W, np.float32)
    low_W = np.asarray(low_W, np.float32)
    h_W = np.asarray(h_W, np.float32)

    in_maps, G, corr = _host_precompute(
        x, lower_lp, upper_lp, up_W, low_W, h_W)

    if "nc" not in _compiled:
        _compiled["nc"] = _build_nc()
    nc = _compiled["nc"]

    trace = os.environ.get("AIRTNN_TRACE", "0") == "1"
    res = run_bass_kernel_spmd(nc, in_maps, list(range(NCORES)), trace=trace)
    LAST_RESULTS = res

    # out[b, d*R + m, o] = G * out_t_d[o + 64*b, m] + corr[d*R + m, o]
    out = np.empty((B, N, C), np.float32)
    for d in range(NCORES):
        ot = res.results[d]["out_t"]  # [C2, R] fp32
        for b in range(B):
            out[b, d * R:(d + 1) * R, :] = (
                ot[b * C:(b + 1) * C, :].T) * G + corr[d * R:(d + 1) * R, :]
    return out
